# revision 16
# baseline (speedup 1.0000x reference)
"""Trainium2 Bass kernel for a 2-layer GCN (GCNConv -> ReLU -> GCNConv -> sigmoid head).

Strategy (8 NeuronCores):
  - Node sharding: core c owns nodes [c*12500, (c+1)*12500), padded to 12544 = 98*128.
  - Edges are assigned to the core that owns their dst node (so segment-sums are local).
  - GCN algebra: agg[i] = dis[i] * sum_{e: dst=i} (dis*h)[src_e] + dis[i]*(dis[i]*h[i]) + b,
    with hp = dis*h stored once per node (bf16), so the self term is dis*hp too.
  - Per layer: local dense matmul h = x@W, scale by dis -> hp (bf16, 128-col rows = 256B
    for dma_gather's 256B elem constraint), AllGather the bf16 feature table in 4 chunks
    (chunk q = quarter q of every core's rows, so int16 gather indices fit 25088 rows),
    then gather source rows per edge-slot with dma_gather (2048 idxs per call = 16 blocks
    of 128 slots) and segment-sum via one-hot matmuls on the tensor engine.
  - Slots are grouped into cells (chunk, dst-tile); each cell gets ceil(maxcount/128)
    blocks (max across cores: the program is SPMD). One-hot [slot -> dst-within-tile]
    built on DVE (bf16 is_equal vs iota), psum accumulates across a cell's blocks,
    chunks accumulate into acc_sb.
"""

import numpy as np
import ml_dtypes

P = 128


class Cfg:
    def __init__(self, n_nodes, n_loc_real, nt, in_c, hid, nchunk, callb=16):
        self.C = 8
        self.N = n_nodes
        self.NLOC_REAL = n_loc_real           # real nodes per core
        self.NT = nt                          # node tiles per core
        self.NLOC = nt * P                    # padded nodes per core
        self.NTAB = self.C * self.NLOC        # global table rows
        self.IN_C = in_c
        self.HID = hid
        self.TABW = 128                       # table row width (bf16) -> 256B rows
        self.NCHUNK = nchunk
        self.CR = self.NTAB // nchunk         # chunk rows (must be < 32768)
        assert self.CR * nchunk == self.NTAB and self.CR < 32768
        self.CALLB = callb                    # blocks per dma_gather call
        self.CALL = callb * P                 # idxs per call


def full_cfg():
    return Cfg(n_nodes=100000, n_loc_real=12500, nt=98, in_c=128, hid=64, nchunk=4, callb=8)


def _prep(cfg, x, edge_index, W1, b1, W2, b2, Wl, bl):
    """Host-side sharding/partitioning. Returns (in_maps, meta)."""
    C, NT, NLOC, NLOC_REAL, NCHUNK = cfg.C, cfg.NT, cfg.NLOC, cfg.NLOC_REAL, cfg.NCHUNK
    CALL, CALLB = cfg.CALL, cfg.CALLB
    src = np.asarray(edge_index[0], dtype=np.int64)
    dst = np.asarray(edge_index[1], dtype=np.int64)
    core = dst // NLOC_REAL
    dst_local = dst - core * NLOC_REAL
    tile = dst_local // P
    dst_rel = dst_local - tile * P

    Q = NLOC // NCHUNK
    src_core = src // NLOC_REAL
    src_l = src % NLOC_REAL
    chunk = src_l // Q
    idx_in_chunk = src_core * Q + (src_l - chunk * Q)

    # shared (SPMD) block structure: per cell (chunk, tile), blocks = ceil(max_c cnt/128)
    key = (core * NCHUNK + chunk) * NT + tile
    counts = np.bincount(key, minlength=C * NCHUNK * NT).reshape(C, NCHUNK, NT)
    Bcell = np.ceil(counts.max(axis=0) / P).astype(np.int64)      # [NCHUNK, NT]
    for t in range(NT):                                            # every tile aggregates
        if Bcell[:, t].sum() == 0:
            Bcell[0, t] = 1
    block_off = np.zeros((NCHUNK, NT + 1), dtype=np.int64)
    np.cumsum(Bcell, axis=1, out=block_off[:, 1:])
    NBLK_ch = block_off[:, -1]                                     # blocks per chunk
    base_bg = np.concatenate([[0], np.cumsum(NBLK_ch)])
    NBLK = int(base_bg[-1])
    blocks = [[(t, b) for t in range(NT) for b in range(int(Bcell[ch, t]))]
              for ch in range(NCHUNK)]
    ncalls = [int(-(-NBLK_ch[ch] // CALLB)) for ch in range(NCHUNK)]
    nvalid = [[int(min(CALL, NBLK_ch[ch] * P - c * CALL)) for c in range(ncalls[ch])]
              for ch in range(NCHUNK)]
    NCOL = sum(ncalls) * (CALL // 16)
    meta = {"Bcell": Bcell, "blocks": blocks, "ncalls": ncalls, "nvalid": nvalid,
            "NBLK": NBLK, "NBLK_ch": [int(v) for v in NBLK_ch], "NCOL": NCOL,
            "base_bg": [int(v) for v in base_bg[:-1]]}

    # per-edge rank within its (core, chunk, tile) cell
    order = np.argsort(key, kind="stable")
    key_s = key[order]
    cell_start = np.zeros(C * NCHUNK * NT + 1, dtype=np.int64)
    np.cumsum(counts.reshape(-1), out=cell_start[1:])
    rank = np.arange(len(src)) - cell_start[key_s]

    in_maps = []
    for c in range(C):
        mask = key_s // (NCHUNK * NT) == c
        sel = order[mask]
        ch_e = chunk[sel]
        t_e = tile[sel]
        r_sel = rank[mask]
        blk = block_off[ch_e, t_e] + r_sel // P                     # block within chunk
        pos = r_sel % P
        slot = blk * P + pos                                        # slot within chunk

        idx16 = [np.zeros(int(NBLK_ch[ch]) * P, dtype=np.int16) for ch in range(NCHUNK)]
        dstrel = np.full((P, NBLK), -1.0, dtype=np.float32)
        for ch in range(NCHUNK):
            m = ch_e == ch
            idx16[ch][slot[m]] = idx_in_chunk[sel][m].astype(np.int16)
            dstrel[pos[m], base_bg[ch] + blk[m]] = dst_rel[sel][m].astype(np.float32)

        cols = []
        for ch in range(NCHUNK):
            for call in range(ncalls[ch]):
                seg = idx16[ch][call * CALL:(call + 1) * CALL]
                if len(seg) < CALL:
                    seg = np.concatenate([seg, np.full(CALL - len(seg), -1, np.int16)])
                w = seg.reshape(-1, 16).T                            # [16, CALL//16]
                cols.append(np.tile(w, (8, 1)))                      # [128, CALL//16]
        gidx = np.ascontiguousarray(np.concatenate(cols, axis=1))
        assert gidx.shape == (P, NCOL)

        m = {}
        xl = np.zeros((P, NLOC), dtype=np.float32)
        xl[:, :NLOC_REAL] = np.asarray(x[c * NLOC_REAL:(c + 1) * NLOC_REAL], np.float32).T
        m["xT"] = np.ascontiguousarray(xl)
        cnt = np.bincount(dst_local[core == c], minlength=NLOC).astype(np.float32) + 1.0
        m["degf"] = np.ascontiguousarray(cnt.reshape(NT, P).T)
        m["gidx"] = gidx
        m["dstrel"] = np.ascontiguousarray(dstrel)
        m["iota"] = np.ascontiguousarray(
            np.tile(np.arange(P, dtype=np.float32), (P, 1)).astype(ml_dtypes.bfloat16))
        m["identm"] = np.eye(P, dtype=np.float32)
        m["W1"] = np.asarray(W1, np.float32)
        m["W2"] = np.asarray(W2, np.float32).astype(ml_dtypes.bfloat16)
        m["b1b"] = np.ascontiguousarray(np.tile(np.asarray(b1, np.float32)[None, :], (P, 1)))
        m["b2b"] = np.ascontiguousarray(np.tile(np.asarray(b2, np.float32)[None, :], (P, 1)))
        m["Wlb"] = np.ascontiguousarray(np.tile(np.asarray(Wl, np.float32)[:, 0][None, :], (P, 1)))
        in_maps.append(m)
    return in_maps, meta


def _program(cfg, meta, bl_value, no_gather=False, linearize=False):
    from concourse import bass, bacc, mybir
    import concourse.tile as tile

    f32 = mybir.dt.float32
    bf16 = mybir.dt.bfloat16
    i16 = mybir.dt.int16
    AF = mybir.ActivationFunctionType
    OP = mybir.AluOpType

    NT, NLOC, HID, TABW, CR = cfg.NT, cfg.NLOC, cfg.HID, cfg.TABW, cfg.CR
    NCHUNK, CALL, CALLB = cfg.NCHUNK, cfg.CALL, cfg.CALLB
    Bcell, blocks, ncalls = meta["Bcell"], meta["blocks"], meta["ncalls"]
    nvalid, NBLK, NBLK_ch = meta["nvalid"], meta["NBLK"], meta["NBLK_ch"]
    NCOL, base_bg = meta["NCOL"], meta["base_bg"]
    Q = NLOC // NCHUNK
    rg = [list(range(cfg.C))]
    # first chunk contributing blocks for each tile (acc copy vs add)
    first_ch = [next(ch for ch in range(NCHUNK) if Bcell[ch, t] > 0) for t in range(NT)]

    nc = bacc.Bacc("TRN2", target_bir_lowering=False, debug=False,
                   num_devices=cfg.C, num_swdge_queues=4)
    xT_d = nc.dram_tensor("xT", [P, NLOC], f32, kind="ExternalInput")
    degf_d = nc.dram_tensor("degf", [P, NT], f32, kind="ExternalInput")
    gidx_d = nc.dram_tensor("gidx", [P, NCOL], i16, kind="ExternalInput")
    dstrel_d = nc.dram_tensor("dstrel", [P, NBLK], f32, kind="ExternalInput")
    iota_d = nc.dram_tensor("iota", [P, P], bf16, kind="ExternalInput")
    identm_d = nc.dram_tensor("identm", [P, P], f32, kind="ExternalInput")
    W1_d = nc.dram_tensor("W1", [cfg.IN_C, HID], f32, kind="ExternalInput")
    W2_d = nc.dram_tensor("W2", [HID, HID], bf16, kind="ExternalInput")
    b1b_d = nc.dram_tensor("b1b", [P, HID], f32, kind="ExternalInput")
    b2b_d = nc.dram_tensor("b2b", [P, HID], f32, kind="ExternalInput")
    Wlb_d = nc.dram_tensor("Wlb", [P, HID], f32, kind="ExternalInput")
    out_d = nc.dram_tensor("out", [NT, P], f32, kind="ExternalOutput")

    hloc = [nc.dram_tensor(f"h{l}loc", [NLOC, TABW], bf16) for l in (1, 2)]
    tabs = [nc.dram_tensor(f"tab{l}", [cfg.NTAB, TABW], bf16, addr_space="Shared")
            for l in (1, 2)]

    with tile.TileContext(nc, linearize=linearize) as tc:
        from contextlib import ExitStack
        with ExitStack() as ctx:
            const = ctx.enter_context(tc.tile_pool(name="const", bufs=1))
            persist = ctx.enter_context(tc.tile_pool(name="persist", bufs=1))
            tmp = ctx.enter_context(tc.tile_pool(name="tmp", bufs=6))
            ohp = ctx.enter_context(tc.tile_pool(name="ohp", bufs=80))
            psum = ctx.enter_context(tc.tile_pool(name="psum", bufs=4, space="PSUM"))
            psA = ctx.enter_context(tc.tile_pool(name="psA", bufs=2, space="PSUM"))
            psE = ctx.enter_context(tc.tile_pool(name="psE", bufs=2, space="PSUM"))

            nreg = {}
            ident = const.tile([P, P], f32, tag="ident")
            nc.sync.dma_start(out=ident[:], in_=identm_d[:, :])
            iota_t = const.tile([P, P], bf16, tag="iota")
            nc.sync.dma_start(out=iota_t[:], in_=iota_d[:, :])
            dstrel_t = const.tile([P, NBLK], f32, tag="dstrel")
            nc.sync.dma_start(out=dstrel_t[:], in_=dstrel_d[:, :])
            W1_t = const.tile([cfg.IN_C, HID], f32, tag="W1")
            nc.sync.dma_start(out=W1_t[:], in_=W1_d[:, :])
            W2_t = const.tile([HID, HID], bf16, tag="W2")
            nc.sync.dma_start(out=W2_t[:], in_=W2_d[:, :])
            b1_t = const.tile([P, HID], f32, tag="b1")
            nc.sync.dma_start(out=b1_t[:], in_=b1b_d[:, :])
            b2_t = const.tile([P, HID], f32, tag="b2")
            nc.sync.dma_start(out=b2_t[:], in_=b2b_d[:, :])
            Wl_t = const.tile([P, HID], f32, tag="Wl")
            nc.sync.dma_start(out=Wl_t[:], in_=Wlb_d[:, :])
            bl_t = const.tile([P, 1], f32, tag="bl")
            nc.vector.memset(bl_t[:], float(bl_value))

            deg = const.tile([P, NT], f32, tag="deg")
            nc.sync.dma_start(out=deg[:], in_=degf_d[:, :])
            selfw = const.tile([P, NT], f32, tag="selfw")
            nc.vector.reciprocal(out=selfw[:], in_=deg[:])
            dis = const.tile([P, NT], f32, tag="dis")
            nc.scalar.activation(out=dis[:], in_=selfw[:], func=AF.Sqrt)

            hp_sb = persist.tile([P, NT * TABW], bf16, tag="hp_sb")
            acc_sb = persist.tile([P, NT * HID], f32, tag="acc_sb")
            zT_sb = persist.tile([HID, NT * P], bf16, tag="zT_sb")
            y_sb = persist.tile([P, NT], f32, tag="y_sb")
            # zero the bf16 table pad columns once (cols HID..TABW of each tile row)
            nc.vector.memset(hp_sb[:], 0.0)

            # AllGather issue points: after the tile covering the end of quarter q
            ag_after = [min(NT - 1, (Q * (q + 1) + P - 1) // P - 1) for q in range(NCHUNK)]

            # last chunk contributing blocks for each tile (tile-complete trigger)
            last_ch = [max(ch for ch in range(NCHUNK) if Bcell[ch, t] > 0)
                       for t in range(NT)]
            ag_state = {}

            def emit_ag(l, q):
                nc.gpsimd.collective_compute(
                    "AllGather", mybir.AluOpType.bypass, replica_groups=rg,
                    ins=[hloc[l - 1][q * Q:(q + 1) * Q, :]],
                    outs=[tabs[l - 1][q * CR:(q + 1) * CR, :]])

            def emit_lA_tile(l, t, xT_t=None):
                """h = in @ W for one tile; hp = dis*h (bf16); DMA row block to hloc.
                Fires the chunk AllGather once every tile of that quarter is written."""
                W_t = W1_t if l == 1 else W2_t
                ps = psA.tile([P, HID], f32, tag="psA")
                lhsT = (xT_t[:, t * P:(t + 1) * P] if l == 1
                        else zT_sb[:, t * P:(t + 1) * P])
                nc.tensor.matmul(out=ps[:], lhsT=lhsT, rhs=W_t[:], start=True, stop=True)
                nc.scalar.activation(out=hp_sb[:, t * TABW:t * TABW + HID], in_=ps[:],
                                     func=AF.Copy, scale=dis[:, t:t + 1])
                nc.sync.dma_start(out=hloc[l - 1][t * P:(t + 1) * P, :],
                                  in_=hp_sb[:, t * TABW:(t + 1) * TABW])
                done = ag_state.setdefault(l, set())
                done.add(t)
                for q in range(NCHUNK):
                    if (l, q) not in ag_state and all(
                            tt in done for tt in range(ag_after[q] + 1)):
                        ag_state[(l, q)] = True
                        emit_ag(l, q)

            def emit_post_tile(l, t):
                """agg = dis*acc + dis*hp + b; l1: relu+transpose into zT; l2: head."""
                b_t = b1_t if l == 1 else b2_t
                t1 = tmp.tile([P, HID], f32, tag="t1")
                nc.vector.tensor_scalar(out=t1[:], in0=acc_sb[:, t * HID:(t + 1) * HID],
                                        scalar1=dis[:, t:t + 1], scalar2=None,
                                        op0=OP.mult)
                t2 = tmp.tile([P, HID], f32, tag="t2")
                nc.scalar.activation(out=t2[:], in_=hp_sb[:, t * TABW:t * TABW + HID],
                                     func=AF.Copy, scale=dis[:, t:t + 1])
                nc.vector.tensor_tensor(out=t1[:], in0=t1[:], in1=t2[:], op=OP.add)
                nc.vector.tensor_tensor(out=t1[:], in0=t1[:], in1=b_t[:], op=OP.add)
                if l == 1:
                    z = tmp.tile([P, HID], f32, tag="z")
                    nc.vector.tensor_scalar(out=z[:], in0=t1[:], scalar1=0.0,
                                            scalar2=None, op0=OP.max)
                    pse = psE.tile([HID, P], f32, tag="psE")
                    nc.tensor.transpose(out=pse[:], in_=z[:], identity=ident[:])
                    nc.vector.tensor_copy(out=zT_sb[:, t * P:(t + 1) * P], in_=pse[:])
                else:
                    m = tmp.tile([P, HID], f32, tag="m")
                    nc.vector.tensor_tensor(out=m[:], in0=t1[:], in1=Wl_t[:], op=OP.mult)
                    r = tmp.tile([P, 1], f32, tag="r")
                    nc.vector.tensor_reduce(out=r[:], in_=m[:],
                                            axis=mybir.AxisListType.X, op=OP.add)
                    nc.scalar.activation(out=y_sb[:, t:t + 1], in_=r[:],
                                         func=AF.Sigmoid, bias=bl_t[:, 0:1])

            def layer_agg(l, tile_done):
                """Per chunk: run-gather (CALLB blocks/call) + one-hot segment-sum.
                Calls tile_done(t) right after tile t's last psum chain is drained,
                so the next layer's work pipelines into the tail of this one."""
                with tc.tile_pool(name=f"gath{l}", bufs=5) as gp, \
                     tc.tile_pool(name=f"gidx{l}", bufs=5) as gip:
                    qi = 0
                    ncell = 0
                    pst = None
                    for ch in range(NCHUNK):
                        ps = None
                        for call in range(ncalls[ch]):
                            col = (sum(ncalls[:ch]) + call) * (CALL // 16)
                            gi = gip.tile([P, CALL // 16], i16, tag=f"gi{qi % 4}")
                            nc.sync.dma_start(out=gi[:],
                                              in_=gidx_d[:, col:col + CALL // 16])
                            gf = gp.tile([P, CALLB, TABW], bf16, tag=f"gf{qi % 4}")
                            if no_gather:
                                nc.vector.memset(gf[:], 0.0)
                            else:
                                nv = nvalid[ch][call]
                                nc.gpsimd.dma_gather(
                                    out_ap=gf[:], in_ap=tabs[l - 1][ch * CR:(ch + 1) * CR, :],
                                    idxs_ap=gi[:], num_idxs=CALL,
                                    num_idxs_reg=nreg.setdefault(nv, nc.gpsimd.to_reg(nv)),
                                    elem_size=TABW, queue_num=qi % 4)
                                qi += 1
                            for j in range(CALLB):
                                bg = call * CALLB + j
                                if bg >= NBLK_ch[ch]:
                                    break
                                t, b = blocks[ch][bg]
                                if b == 0:
                                    if ncell % 8 == 0:
                                        pst = psum.tile([P, 8 * HID], f32, tag="psC")
                                    sl = (ncell % 8) * HID
                                    ps = pst[:, sl:sl + HID]
                                    ncell += 1
                                oh = ohp.tile([P, P], bf16, tag="oh")
                                gg = base_bg[ch] + bg
                                nc.vector.tensor_scalar(
                                    out=oh[:], in0=iota_t[:],
                                    scalar1=dstrel_t[:, gg:gg + 1],
                                    scalar2=None, op0=OP.is_equal)
                                nc.tensor.matmul(
                                    out=ps, lhsT=oh[:],
                                    rhs=gf[:, j, 0:HID],
                                    start=(b == 0), stop=(b == int(Bcell[ch, t]) - 1))
                                if b == int(Bcell[ch, t]) - 1:
                                    dstslice = acc_sb[:, t * HID:(t + 1) * HID]
                                    if first_ch[t] == ch:
                                        nc.scalar.copy(out=dstslice, in_=ps)
                                    else:
                                        nc.vector.tensor_tensor(
                                            out=dstslice, in0=dstslice,
                                            in1=ps, op=OP.add)
                                    if last_ch[t] == ch:
                                        tile_done(t)

            def l1_tile_done(t):
                emit_post_tile(1, t)
                emit_lA_tile(2, t)

            with tc.tile_pool(name="xt", bufs=1) as xtp:
                xT_t = xtp.tile([P, NLOC], f32, tag="xT")
                nc.sync.dma_start(out=xT_t[:], in_=xT_d[:, :])
                for t in range(NT):
                    emit_lA_tile(1, t, xT_t)
                layer_agg(1, l1_tile_done)
            layer_agg(2, lambda t: emit_post_tile(2, t))

            psG = psE.tile([NT, P], f32, tag="psE")
            nc.tensor.matmul(out=psG[:], lhsT=y_sb[:, :NT], rhs=ident[:],
                             start=True, stop=True, is_transpose=True)
            og = tmp.tile([NT, P], f32, tag="og")
            nc.scalar.copy(out=og[:], in_=psG[:])
            nc.sync.dma_start(out=out_d[:, :], in_=og[:])
    nc.compile()
    return nc


def kernel(x, edge_index, W1, b1, W2, b2, Wl, bl):
    from concourse.bass_utils import run_bass_kernel_spmd
    cfg = full_cfg()
    in_maps, meta = _prep(cfg, x, edge_index, W1, b1, W2, b2, Wl, bl)
    nc = _program(cfg, meta, float(np.asarray(bl).reshape(-1)[0]))
    res = run_bass_kernel_spmd(nc, in_maps, list(range(cfg.C)))
    outs = []
    for c in range(cfg.C):
        o = np.asarray(res.results[c]["out"], dtype=np.float32).reshape(cfg.NLOC)
        outs.append(o[:cfg.NLOC_REAL])
    return np.concatenate(outs).reshape(cfg.N, 1).astype(np.float32)
